# revision 31
# baseline (speedup 1.0000x reference)
"""Trainium2 Bass kernel for MultiHeadSelfAttention (B=4, L=2048, H=1024, NH=16).

Sharding: 8 cores = 4 batches x 2 head-groups (8 heads each).

Per core, one fused pipeline (single PSUM pool, no phase barriers):
- x is shipped as fp8e4 hi+lo residual pair at a common x16 scale; q/k
  projections use the hi part only (DoubleRow fp8 matmuls), the v
  projection accumulates hi*hi + lo*hi + hi*lo DoubleRow passes in one
  PSUM group (common product scale 256) for ~bf16 accuracy at fp8 speed.
- q/k biases fused into the PSUM->SBUF copy on VectorE (tensor_scalar,
  per-partition bias); bv folded into the output bias on host (softmax
  rows sum to one).
- v stored per head as 64 v-columns + 1 ones-column so each AV matmul
  yields the numerator in rows 0:64 and the softmax denominator in
  row 64 of the same PSUM tile.
- S^T-layout attention, K=64 row-tiled S matmuls (head pair occupies
  disjoint PE row-groups), exp on ScalarE, mask multiply on VectorE
  (bf16 2x), reciprocal on VectorE broadcast across partitions on
  GpSimd, final divide on VectorE.
- v projection is woven into head 0's key loop (AV of tile j needs v of
  tile j); q/k projections of pair p run at head p; the first query
  half's out projection is spread through the second half's head loops.
- bf16 output; host sums the two head-group partials in f32.
"""

import os
import sys

os.environ.setdefault("JAX_PLATFORMS", "")
try:
    import concourse.bass as bass  # noqa: F401
except ImportError:
    sys.path.insert(0, "/opt/trn_rl_repo")

import numpy as np

import concourse.bass as bass  # noqa: F811
import concourse.mybir as mybir
import concourse.tile as tile
from concourse import bacc
from concourse import bass_utils

BF16 = mybir.dt.bfloat16
F32 = mybir.dt.float32
FP8 = mybir.dt.float8e4

B, L, H = 4, 2048, 1024
NH, HD = 16, 64
NCORES = 8
HPC = NH // 2          # heads per core = 8
CPC = H // 2           # channels per core = 512
PAIRS = HPC // 2       # 4 head pairs per core
LT = L // 128          # 16 token tiles
KC8 = H // 256         # 4 fp8 DoubleRow contraction chunks
VW = HD + 1            # 65 v columns per head (64 v + ones)
WPRE = 16.0            # common fp8 scale on x and W tensors
SC = 0.125 / (WPRE ** 4)   # exp scale: 1/sqrt(HD) / (16^2 q * 16^2 k)
VPOST = 1.0 / (WPRE * WPRE)  # v copy-out scale


def build_nc(repeats=1):
    nc = bacc.Bacc("TRN2", target_bir_lowering=False, debug=False,
                   num_devices=NCORES)

    x8h = nc.dram_tensor("x8h", [H, L], FP8, kind="ExternalInput").ap()
    x8l = nc.dram_tensor("x8l", [H, L], FP8, kind="ExternalInput").ap()
    wq8h = nc.dram_tensor("wq8h", [H, CPC], FP8, kind="ExternalInput").ap()
    wq8l = nc.dram_tensor("wq8l", [H, CPC], FP8, kind="ExternalInput").ap()
    wk8h = nc.dram_tensor("wk8h", [H, CPC], FP8, kind="ExternalInput").ap()
    wk8l = nc.dram_tensor("wk8l", [H, CPC], FP8, kind="ExternalInput").ap()
    wv8h = nc.dram_tensor("wv8h", [H, CPC], FP8, kind="ExternalInput").ap()
    wv8l = nc.dram_tensor("wv8l", [H, CPC], FP8, kind="ExternalInput").ap()
    woT = nc.dram_tensor("woT", [CPC, H], BF16, kind="ExternalInput").ap()
    qkb = nc.dram_tensor("qkb", [128, 2 * PAIRS], F32,
                         kind="ExternalInput").ap()
    maskT = nc.dram_tensor("maskT", [L, L], BF16, kind="ExternalInput").ap()
    out = nc.dram_tensor("out", [L, H], BF16, kind="ExternalOutput").ap()

    with tile.TileContext(nc) as tc:
        for _ in range(repeats):
            mhsa_body(tc, x8h, x8l, wq8h, wq8l, wk8h, wk8l, wv8h,
                      wv8l, woT, qkb, maskT, out)
    nc.compile()
    return nc


def mhsa_body(tc, x8h, x8l, wq8h, wq8l, wk8h, wk8l, wv8h, wv8l, woT,
              qkb, maskT, out):
    nc = tc.nc
    Exp = mybir.ActivationFunctionType.Exp
    mult = mybir.AluOpType.mult
    add = mybir.AluOpType.add
    DR = mybir.MatmulPerfMode.DoubleRow

    # [H, *] -> [p, kc256, half, *]: row = kc*256 + half*128 + p
    def r8(t):
        return t.rearrange("(kc half p) c -> p kc half c", p=128, half=2)

    x8h_r, x8l_r = r8(x8h), r8(x8l)
    wqh_r, wql_r = r8(wq8h), r8(wq8l)
    wkh_r, wkl_r = r8(wk8h), r8(wk8l)
    wvh_r, wvl_r = r8(wv8h), r8(wv8l)
    wo_r = woT.rearrange("(kc p) c -> p kc c", p=128)
    mask_r = maskT.rearrange("(jt p) i -> p jt i", p=128)
    out_r = out.rearrange("(tt p) c -> p tt c", p=128)

    import contextlib
    ctx = contextlib.ExitStack()
    with ctx:
        consts = ctx.enter_context(tc.tile_pool(name="consts", bufs=1))
        wpool = ctx.enter_context(tc.tile_pool(name="weights", bufs=1))
        xpool = ctx.enter_context(tc.tile_pool(name="xpool", bufs=1))
        qkv_sb = ctx.enter_context(tc.tile_pool(name="qkv_sb", bufs=1))
        ao_pool = ctx.enter_context(tc.tile_pool(name="ao", bufs=1))
        mpool = ctx.enter_context(tc.tile_pool(name="mask", bufs=1))
        ppool = ctx.enter_context(tc.tile_pool(name="ptiles", bufs=4))
        rpool = ctx.enter_context(tc.tile_pool(name="rec", bufs=1))
        o_sbp = ctx.enter_context(tc.tile_pool(name="o_sb", bufs=2))
        ps = ctx.enter_context(tc.tile_pool(name="ps", bufs=1, space="PSUM"))

        qkb_sb = consts.tile([128, 2 * PAIRS], F32, tag="qkb")
        qT_sb = qkv_sb.tile([128, PAIRS, L], BF16, tag="qT")
        kT_sb = qkv_sb.tile([128, PAIRS, L], BF16, tag="kT")
        # per head: 64 v columns then 1 ones column -> AV matmul yields
        # numerator in rows 0:64 and the softmax denominator in row 64
        v_sb = qkv_sb.tile([128, LT, HPC * VW], BF16, tag="v")
        aoT_sb = ao_pool.tile([128, PAIRS, L], BF16, tag="aoT")
        mask_sb = mpool.tile([128, LT, L], BF16, tag="mask")
        v_aug = v_sb[:].rearrange("p t (h w) -> p t h w", w=VW)

        # ---- input DMAs (q/k path, then v path, then mask) ----
        wqh_sb = wpool.tile([128, KC8, 2, CPC], FP8, tag="wqh")
        wql_sb = wpool.tile([128, KC8, 2, CPC], FP8, tag="wql")
        wkh_sb = wpool.tile([128, KC8, 2, CPC], FP8, tag="wkh")
        wkl_sb = wpool.tile([128, KC8, 2, CPC], FP8, tag="wkl")
        nc.sync.dma_start(wqh_sb[:], wqh_r)
        nc.sync.dma_start(wql_sb[:], wql_r)
        nc.sync.dma_start(wkh_sb[:], wkh_r)
        nc.sync.dma_start(wkl_sb[:], wkl_r)
        x8h_sb = xpool.tile([128, KC8, 2, L], FP8, tag="x8h")
        for kc in range(KC8):
            nc.sync.dma_start(x8h_sb[:, kc], x8h_r[:, kc])
        nc.sync.dma_start(qkb_sb[:], qkb)
        x8l_sb = xpool.tile([128, KC8, 2, L], FP8, tag="x8l")
        for kc in range(KC8):
            nc.sync.dma_start(x8l_sb[:, kc], x8l_r[:, kc])
        wvh_sb = wpool.tile([128, KC8, 2, CPC], FP8, tag="wvh")
        wvl_sb = wpool.tile([128, KC8, 2, CPC], FP8, tag="wvl")
        nc.sync.dma_start(wvh_sb[:], wvh_r)
        nc.sync.dma_start(wvl_sb[:], wvl_r)
        for jt in range(LT):
            nc.sync.dma_start(mask_sb[:, jt], mask_r[:, jt])
        wo_sb = wpool.tile([128, PAIRS, H], BF16, tag="wo")
        nc.sync.dma_start(wo_sb[:], wo_r)
        nc.any.memset(v_aug[:, :, :, HD], 1.0)

        def qkproj(mc):
            # q/k of token-half 0 first: head 2mc's first S tiles need them
            for nh in range(2):
                tok = slice(nh * 1024, (nh + 1) * 1024)
                for is_k, whi, wlo, dst in ((0, wqh_sb, wql_sb, qT_sb),
                                            (1, wkh_sb, wkl_sb, kT_sb)):
                    psqk = ps.tile([128, 1024], F32, tag="nd", name="psqk",
                                   bufs=2)
                    passes = ((x8h_sb, whi), (x8h_sb, wlo), (x8l_sb, whi))
                    for pi, (xs, ws) in enumerate(passes):
                        for kc in range(KC8):
                            for hf in range(2):
                                nc.tensor.matmul(
                                    psqk[:, hf * 512:(hf + 1) * 512],
                                    ws[:, kc, :, mc * 128:(mc + 1) * 128],
                                    xs[:, kc, :,
                                       nh * 1024 + hf * 512:
                                       nh * 1024 + (hf + 1) * 512],
                                    start=(pi == 0 and kc == 0),
                                    stop=(pi == 2 and kc == KC8 - 1),
                                    perf_mode=DR,
                                )
                    nc.vector.tensor_scalar(
                        dst[:, mc, tok], psqk[:],
                        qkb_sb[:, is_k * PAIRS + mc:is_k * PAIRS + mc + 1],
                        None, add)

        def vproj(t):
            # hi*hi + lo*hi + hi*lo DoubleRow passes, one PSUM group
            psv = ps.tile([128, 512], F32, tag="nd", name="psv", bufs=2,
                          padded_shape=[128, 1024])
            passes = ((x8h_sb, wvh_sb), (x8h_sb, wvl_sb), (x8l_sb, wvh_sb))
            for pi, (xs, ws) in enumerate(passes):
                for kc in range(KC8):
                    nc.tensor.matmul(
                        psv[:],
                        xs[:, kc, :, t * 128:(t + 1) * 128],
                        ws[:, kc, :, :],
                        start=(pi == 0 and kc == 0),
                        stop=(pi == 2 and kc == KC8 - 1),
                        perf_mode=DR,
                    )
            nc.vector.tensor_scalar(
                v_aug[:, t, :, 0:HD],
                psv[:].rearrange("p (h d) -> p h d", d=HD),
                VPOST, None, mult)

        pso_live = {}

        def outproj_mm(tt, hf, kc):
            if kc == 0:
                pso_live[(tt, hf)] = ps.tile(
                    [128, 512], F32, tag="nd", name="pso", bufs=2,
                    padded_shape=[128, 1024])
            pso = pso_live[(tt, hf)]
            nc.tensor.matmul(
                pso[:],
                aoT_sb[:, kc, tt * 128:(tt + 1) * 128],
                wo_sb[:, kc, hf * 512:(hf + 1) * 512],
                start=(kc == 0), stop=(kc == PAIRS - 1),
            )
            if kc == PAIRS - 1:
                ob = o_sbp.tile([128, 512], BF16, tag="ob")
                nc.vector.tensor_copy(ob[:], pso[:])
                nc.sync.dma_start(
                    out_r[:, tt, hf * 512:(hf + 1) * 512], ob[:])
                del pso_live[(tt, hf)]

        def outproj(tt):
            for hf in range(2):
                for kc in range(PAIRS):
                    outproj_mm(tt, hf, kc)

        # ---------------- attention + projections ----------------
        for ih in range(2):  # query halves
            for h in range(HPC):
                p = h // 2
                rb = (h % 2) * 64
                if ih == 0 and h < PAIRS:
                    qkproj(h)  # pair h, >=1 head of slack before first use
                nd = ps.tile([128, 1024], F32, tag="nd", name=f"nd{ih}_{h}",
                             bufs=2)
                for j in range(LT):
                    s = ps.tile([128, 1024], F32, tag="s", name="s", bufs=2)
                    for c in range(2):
                        q0 = ih * 1024 + c * 512
                        nc.tensor.matmul(
                            s[:, c * 512:(c + 1) * 512],
                            kT_sb[rb:rb + 64, p, j * 128:(j + 1) * 128],
                            qT_sb[rb:rb + 64, p, q0:q0 + 512],
                            start=True, stop=True,
                        )
                    if ih == 0 and h == 0:
                        vproj(j)  # AV of tile j consumes v of tile j
                    if ih == 1 and 8 <= j < 16:
                        # first query half's out projection: tiles 2h,2h+1
                        # spread 2 matmuls per key-tile step
                        for i in range(2):
                            m = 2 * (j - 8) + i
                            outproj_mm(2 * h + m // 8, (m % 8) // 4, m % 4)
                    pm = ppool.tile([128, 1024], BF16, tag="pm")
                    nc.scalar.activation(pm[:], s[:], Exp, scale=SC)
                    nc.vector.tensor_tensor(
                        pm[:], pm[:],
                        mask_sb[:, j, ih * 1024:(ih + 1) * 1024], mult)
                    for c in range(2):
                        nc.tensor.matmul(
                            nd[0:VW, c * 512:(c + 1) * 512],
                            v_sb[:, j, h * VW:(h + 1) * VW],
                            pm[:, c * 512:(c + 1) * 512],
                            start=(j == 0), stop=(j == LT - 1),
                        )
                recs = [rpool.tile([128, 512], F32, tag="rec",
                                   name=f"rec{ih}_{h}_{c}") for c in range(2)]
                for c in range(2):
                    nc.vector.reciprocal_approx_fast(
                        recs[c][0:1, :], nd[64:65, c * 512:(c + 1) * 512])
                for c in range(2):
                    nc.gpsimd.partition_broadcast(
                        recs[c][0:64, :], recs[c][0:1, :], channels=64)
                for c in range(2):
                    nc.vector.tensor_tensor(
                        aoT_sb[rb:rb + 64, p,
                               ih * 1024 + c * 512:ih * 1024 + (c + 1) * 512],
                        nd[0:64, c * 512:(c + 1) * 512],
                        recs[c][0:64, :],
                        mult)
        # tail: second half's own out projection
        for tt in range(8, 16):
            outproj(tt)


_NC_CACHE = None


def get_nc():
    global _NC_CACHE
    if _NC_CACHE is None:
        _NC_CACHE = build_nc()
    return _NC_CACHE


def make_in_maps(x, attn_mask, Wq, bq, Wk, bk, Wv, bv, Wo, bo):
    import ml_dtypes
    bf = ml_dtypes.bfloat16
    f8 = mybir.dt.np(FP8)
    x = np.asarray(x, np.float32)
    attn_mask = np.asarray(attn_mask)
    in_maps = []
    for core in range(NCORES):
        b, pg = divmod(core, 2)
        cs = slice(pg * CPC, (pg + 1) * CPC)
        xT = np.ascontiguousarray(x[b].T) * WPRE  # [H, L], x16
        x8h = xT.astype(f8)
        x8l = (xT - x8h.astype(np.float32)).astype(f8)
        wvT = np.ascontiguousarray(
            np.asarray(Wv, np.float32)[cs, :].T) * WPRE
        wv8h = wvT.astype(f8)
        wv8l = (wvT - wv8h.astype(np.float32)).astype(f8)
        wqT = np.ascontiguousarray(
            np.asarray(Wq, np.float32)[cs, :].T) * WPRE
        wq8h = wqT.astype(f8)
        wq8l = (wqT - wq8h.astype(np.float32)).astype(f8)
        wkT = np.ascontiguousarray(
            np.asarray(Wk, np.float32)[cs, :].T) * WPRE
        wk8h = wkT.astype(f8)
        wk8l = (wkT - wk8h.astype(np.float32)).astype(f8)
        m = {
            "x8h": x8h,
            "x8l": x8l,
            "wq8h": wq8h, "wq8l": wq8l,
            "wk8h": wk8h, "wk8l": wk8l,
            "wv8h": wv8h,
            "wv8l": wv8l,
            "woT": np.ascontiguousarray(
                np.asarray(Wo, np.float32)[:, cs].T).astype(bf),
            "maskT": np.ascontiguousarray(attn_mask[b, 0].T).astype(bf),
        }
        qkb = np.zeros((128, 2 * PAIRS), np.float32)
        for mc in range(PAIRS):
            chs = slice(pg * CPC + mc * 128, pg * CPC + (mc + 1) * 128)
            qkb[:, mc] = np.asarray(bq, np.float32)[chs] * (WPRE * WPRE)
            qkb[:, PAIRS + mc] = (np.asarray(bk, np.float32)[chs]
                                  * (WPRE * WPRE))
        m["qkb"] = qkb
        in_maps.append(m)
    return in_maps


def gather(results, bo_eff):
    out = np.empty((B, L, H), np.float32)
    for b in range(B):
        out[b] = (results[2 * b]["out"].astype(np.float32)
                  + results[2 * b + 1]["out"].astype(np.float32)
                  + bo_eff)
    return out


def kernel(x, attn_mask, Wq, bq, Wk, bk, Wv, bv, Wo, bo):
    nc = get_nc()
    in_maps = make_in_maps(x, attn_mask, Wq, bq, Wk, bk, Wv, bv, Wo, bo)
    res = bass_utils.run_bass_kernel_spmd(nc, in_maps,
                                          core_ids=list(range(NCORES)))
    bo_eff = (np.asarray(bo, np.float32)
              + np.asarray(bv, np.float32) @ np.asarray(Wo, np.float32).T)
    return gather(res.results, bo_eff)


# revision 35
# speedup vs baseline: 1.0421x; 1.0421x over previous
"""Trainium2 Bass kernel for MultiHeadSelfAttention (B=4, L=2048, H=1024, NH=16).

Sharding: 8 cores = 4 batches x 2 head-groups (8 heads each).

Per core, one fused pipeline (single PSUM pool, no phase barriers):
- x is shipped as fp8e4 hi+lo residual pair at a common x16 scale; q/k
  projections use the hi part only (DoubleRow fp8 matmuls), the v
  projection accumulates hi*hi + lo*hi + hi*lo DoubleRow passes in one
  PSUM group (common product scale 256) for ~bf16 accuracy at fp8 speed.
- q/k biases fused into the PSUM->SBUF copy on VectorE (tensor_scalar,
  per-partition bias); bv folded into the output bias on host (softmax
  rows sum to one).
- v stored per head as 64 v-columns + 1 ones-column so each AV matmul
  yields the numerator in rows 0:64 and the softmax denominator in
  row 64 of the same PSUM tile.
- S^T-layout attention, K=64 row-tiled S matmuls (head pair occupies
  disjoint PE row-groups), exp on ScalarE, mask multiply on VectorE
  (bf16 2x), reciprocal on VectorE broadcast across partitions on
  GpSimd, final divide on VectorE.
- v projection is woven into head 0's key loop (AV of tile j needs v of
  tile j); q/k projections of pair p run at head p; the first query
  half's out projection is spread through the second half's head loops.
- bf16 output; host sums the two head-group partials in f32.
"""

import os
import sys

os.environ.setdefault("JAX_PLATFORMS", "")
try:
    import concourse.bass as bass  # noqa: F401
except ImportError:
    sys.path.insert(0, "/opt/trn_rl_repo")

import numpy as np

import concourse.bass as bass  # noqa: F811
import concourse.mybir as mybir
import concourse.tile as tile
from concourse import bacc
from concourse import bass_utils

BF16 = mybir.dt.bfloat16
F32 = mybir.dt.float32
FP8 = mybir.dt.float8e4

B, L, H = 4, 2048, 1024
NH, HD = 16, 64
NCORES = 8
HPC = NH // 2          # heads per core = 8
CPC = H // 2           # channels per core = 512
PAIRS = HPC // 2       # 4 head pairs per core
LT = L // 128          # 16 token tiles
KC8 = H // 256         # 4 fp8 DoubleRow contraction chunks
VW = HD + 1            # 65 v columns per head (64 v + ones)
WPRE = 16.0            # common fp8 scale on x and W tensors
SC = 0.125 / (WPRE ** 4)   # exp scale: 1/sqrt(HD) / (16^2 q * 16^2 k)
VPOST = 1.0 / (WPRE * WPRE)  # v copy-out scale


def build_nc(repeats=1):
    nc = bacc.Bacc("TRN2", target_bir_lowering=False, debug=False,
                   num_devices=NCORES)

    x8h = nc.dram_tensor("x8h", [H, L], FP8, kind="ExternalInput").ap()
    x8l = nc.dram_tensor("x8l", [H, L], FP8, kind="ExternalInput").ap()
    wq8h = nc.dram_tensor("wq8h", [H, CPC], FP8, kind="ExternalInput").ap()
    wq8l = nc.dram_tensor("wq8l", [H, CPC], FP8, kind="ExternalInput").ap()
    wk8h = nc.dram_tensor("wk8h", [H, CPC], FP8, kind="ExternalInput").ap()
    wk8l = nc.dram_tensor("wk8l", [H, CPC], FP8, kind="ExternalInput").ap()
    wv8h = nc.dram_tensor("wv8h", [H, CPC], FP8, kind="ExternalInput").ap()
    wv8l = nc.dram_tensor("wv8l", [H, CPC], FP8, kind="ExternalInput").ap()
    woT = nc.dram_tensor("woT", [CPC, H], BF16, kind="ExternalInput").ap()
    qkb = nc.dram_tensor("qkb", [128, 2 * PAIRS], F32,
                         kind="ExternalInput").ap()
    maskT = nc.dram_tensor("maskT", [L, L], BF16, kind="ExternalInput").ap()
    out = nc.dram_tensor("out", [L, H], BF16, kind="ExternalOutput").ap()

    with tile.TileContext(nc) as tc:
        for _ in range(repeats):
            mhsa_body(tc, x8h, x8l, wq8h, wq8l, wk8h, wk8l, wv8h,
                      wv8l, woT, qkb, maskT, out)
    nc.compile()
    return nc


def mhsa_body(tc, x8h, x8l, wq8h, wq8l, wk8h, wk8l, wv8h, wv8l, woT,
              qkb, maskT, out):
    nc = tc.nc
    Exp = mybir.ActivationFunctionType.Exp
    mult = mybir.AluOpType.mult
    add = mybir.AluOpType.add
    DR = mybir.MatmulPerfMode.DoubleRow

    # [H, *] -> [p, kc256, half, *]: row = kc*256 + half*128 + p
    def r8(t):
        return t.rearrange("(kc half p) c -> p kc half c", p=128, half=2)

    x8h_r, x8l_r = r8(x8h), r8(x8l)
    wqh_r, wql_r = r8(wq8h), r8(wq8l)
    wkh_r, wkl_r = r8(wk8h), r8(wk8l)
    wvh_r, wvl_r = r8(wv8h), r8(wv8l)
    wo_r = woT.rearrange("(kc p) c -> p kc c", p=128)
    mask_r = maskT.rearrange("(jt p) i -> p jt i", p=128)
    out_r = out.rearrange("(tt p) c -> p tt c", p=128)

    import contextlib
    ctx = contextlib.ExitStack()
    with ctx:
        consts = ctx.enter_context(tc.tile_pool(name="consts", bufs=1))
        wpool = ctx.enter_context(tc.tile_pool(name="weights", bufs=1))
        xpool = ctx.enter_context(tc.tile_pool(name="xpool", bufs=1))
        qkv_sb = ctx.enter_context(tc.tile_pool(name="qkv_sb", bufs=1))
        ao_pool = ctx.enter_context(tc.tile_pool(name="ao", bufs=1))
        mpool = ctx.enter_context(tc.tile_pool(name="mask", bufs=1))
        ppool = ctx.enter_context(tc.tile_pool(name="ptiles", bufs=3))
        rpool = ctx.enter_context(tc.tile_pool(name="rec", bufs=1))
        o_sbp = ctx.enter_context(tc.tile_pool(name="o_sb", bufs=2))
        ps = ctx.enter_context(tc.tile_pool(name="ps", bufs=1, space="PSUM"))

        qkb_sb = consts.tile([128, 2 * PAIRS], F32, tag="qkb")
        qT_sb = qkv_sb.tile([128, PAIRS, L], BF16, tag="qT")
        kT_sb = qkv_sb.tile([128, PAIRS, L], BF16, tag="kT")
        # per head: 64 v columns then 1 ones column -> AV matmul yields
        # numerator in rows 0:64 and the softmax denominator in row 64
        v_sb = qkv_sb.tile([128, LT, HPC * VW], BF16, tag="v")
        aoT_sb = ao_pool.tile([128, PAIRS, L], BF16, tag="aoT")
        mask_sb = mpool.tile([128, LT, L], BF16, tag="mask")
        v_aug = v_sb[:].rearrange("p t (h w) -> p t h w", w=VW)

        # ---- input DMAs (q/k path, then v path, then mask) ----
        wqh_sb = wpool.tile([128, KC8, 2, CPC], FP8, tag="wqh")
        wql_sb = wpool.tile([128, KC8, 2, CPC], FP8, tag="wql")
        wkh_sb = wpool.tile([128, KC8, 2, CPC], FP8, tag="wkh")
        wkl_sb = wpool.tile([128, KC8, 2, CPC], FP8, tag="wkl")
        nc.sync.dma_start(wqh_sb[:], wqh_r)
        nc.sync.dma_start(wql_sb[:], wql_r)
        nc.sync.dma_start(wkh_sb[:], wkh_r)
        nc.sync.dma_start(wkl_sb[:], wkl_r)
        x8h_sb = xpool.tile([128, KC8, 2, L], FP8, tag="x8h")
        for kc in range(KC8):
            nc.sync.dma_start(x8h_sb[:, kc], x8h_r[:, kc])
        nc.sync.dma_start(qkb_sb[:], qkb)
        x8l_sb = xpool.tile([128, KC8, 2, L], FP8, tag="x8l")
        for kc in range(KC8):
            nc.sync.dma_start(x8l_sb[:, kc], x8l_r[:, kc])
        wvh_sb = wpool.tile([128, KC8, 2, CPC], FP8, tag="wvh")
        wvl_sb = wpool.tile([128, KC8, 2, CPC], FP8, tag="wvl")
        nc.sync.dma_start(wvh_sb[:], wvh_r)
        nc.sync.dma_start(wvl_sb[:], wvl_r)
        for jt in range(LT):
            nc.sync.dma_start(mask_sb[:, jt], mask_r[:, jt])
        wo_sb = wpool.tile([128, PAIRS, H], BF16, tag="wo")
        nc.sync.dma_start(wo_sb[:], wo_r)
        nc.any.memset(v_aug[:, :, :, HD], 1.0)

        def qkproj(mc):
            # q/k of token-half 0 first: head 2mc's first S tiles need them
            for nh in range(2):
                tok = slice(nh * 1024, (nh + 1) * 1024)
                for is_k, whi, wlo, dst in ((0, wqh_sb, wql_sb, qT_sb),
                                            (1, wkh_sb, wkl_sb, kT_sb)):
                    psqk = ps.tile([128, 1024], F32, tag="nd", name="psqk",
                                   bufs=2)
                    passes = ((x8h_sb, whi), (x8h_sb, wlo), (x8l_sb, whi))
                    for pi, (xs, ws) in enumerate(passes):
                        for kc in range(KC8):
                            for hf in range(2):
                                nc.tensor.matmul(
                                    psqk[:, hf * 512:(hf + 1) * 512],
                                    ws[:, kc, :, mc * 128:(mc + 1) * 128],
                                    xs[:, kc, :,
                                       nh * 1024 + hf * 512:
                                       nh * 1024 + (hf + 1) * 512],
                                    start=(pi == 0 and kc == 0),
                                    stop=(pi == 2 and kc == KC8 - 1),
                                    perf_mode=DR,
                                )
                    nc.vector.tensor_scalar(
                        dst[:, mc, tok], psqk[:],
                        qkb_sb[:, is_k * PAIRS + mc:is_k * PAIRS + mc + 1],
                        None, add)

        def vproj(t):
            # hi*hi + lo*hi + hi*lo DoubleRow passes, one PSUM group
            psv = ps.tile([128, 512], F32, tag="nd", name="psv", bufs=2,
                          padded_shape=[128, 1024])
            passes = ((x8h_sb, wvh_sb), (x8h_sb, wvl_sb), (x8l_sb, wvh_sb))
            for pi, (xs, ws) in enumerate(passes):
                for kc in range(KC8):
                    nc.tensor.matmul(
                        psv[:],
                        xs[:, kc, :, t * 128:(t + 1) * 128],
                        ws[:, kc, :, :],
                        start=(pi == 0 and kc == 0),
                        stop=(pi == 2 and kc == KC8 - 1),
                        perf_mode=DR,
                    )
            nc.vector.tensor_scalar(
                v_aug[:, t, :, 0:HD],
                psv[:].rearrange("p (h d) -> p h d", d=HD),
                VPOST, None, mult)

        pso_live = {}
        ob_live = {}

        def outproj_mm(tt, hf, kc):
            if kc == 0:
                pso_live[(tt, hf)] = ps.tile(
                    [128, 512], F32, tag="nd", name="pso", bufs=2,
                    padded_shape=[128, 1024])
            pso = pso_live[(tt, hf)]
            nc.tensor.matmul(
                pso[:],
                aoT_sb[:, kc, tt * 128:(tt + 1) * 128],
                wo_sb[:, kc, hf * 512:(hf + 1) * 512],
                start=(kc == 0), stop=(kc == PAIRS - 1),
            )
            if kc == PAIRS - 1:
                if hf == 0:
                    ob_live[tt] = o_sbp.tile([128, 1024], BF16, tag="ob",
                                             name="ob")
                ob = ob_live[tt]
                nc.vector.tensor_copy(
                    ob[:, hf * 512:(hf + 1) * 512], pso[:])
                if hf == 1:
                    nc.sync.dma_start(out_r[:, tt, :], ob[:])
                    del ob_live[tt]
                del pso_live[(tt, hf)]

        def outproj(tt):
            for hf in range(2):
                for kc in range(PAIRS):
                    outproj_mm(tt, hf, kc)

        # ---------------- attention + projections ----------------
        for ih in range(2):  # query halves
            for h in range(HPC):
                p = h // 2
                rb = (h % 2) * 64
                if ih == 0 and h < PAIRS:
                    qkproj(h)  # pair h, >=1 head of slack before first use
                nd = ps.tile([128, 1024], F32, tag="nd", name=f"nd{ih}_{h}",
                             bufs=2)
                for j in range(LT):
                    s = ps.tile([128, 1024], F32, tag="s", name="s", bufs=2)
                    for c in range(2):
                        q0 = ih * 1024 + c * 512
                        nc.tensor.matmul(
                            s[:, c * 512:(c + 1) * 512],
                            kT_sb[rb:rb + 64, p, j * 128:(j + 1) * 128],
                            qT_sb[rb:rb + 64, p, q0:q0 + 512],
                            start=True, stop=True,
                        )
                    if ih == 0 and h == 0:
                        vproj(j)  # AV of tile j consumes v of tile j
                    if ih == 1 and 8 <= j < 16:
                        # first query half's out projection: tiles 2h,2h+1
                        # spread 2 matmuls per key-tile step
                        for i in range(2):
                            m = 2 * (j - 8) + i
                            outproj_mm(2 * h + m // 8, (m % 8) // 4, m % 4)
                    pm = ppool.tile([128, 1024], BF16, tag="pm")
                    nc.scalar.activation(pm[:], s[:], Exp, scale=SC)
                    nc.vector.tensor_tensor(
                        pm[:], pm[:],
                        mask_sb[:, j, ih * 1024:(ih + 1) * 1024], mult)
                    for c in range(2):
                        nc.tensor.matmul(
                            nd[0:VW, c * 512:(c + 1) * 512],
                            v_sb[:, j, h * VW:(h + 1) * VW],
                            pm[:, c * 512:(c + 1) * 512],
                            start=(j == 0), stop=(j == LT - 1),
                        )
                recs = [rpool.tile([128, 512], F32, tag="rec",
                                   name=f"rec{ih}_{h}_{c}") for c in range(2)]
                for c in range(2):
                    nc.vector.reciprocal_approx_fast(
                        recs[c][0:1, :], nd[64:65, c * 512:(c + 1) * 512])
                for c in range(2):
                    nc.gpsimd.partition_broadcast(
                        recs[c][0:64, :], recs[c][0:1, :], channels=64)
                for c in range(2):
                    nc.vector.tensor_tensor(
                        aoT_sb[rb:rb + 64, p,
                               ih * 1024 + c * 512:ih * 1024 + (c + 1) * 512],
                        nd[0:64, c * 512:(c + 1) * 512],
                        recs[c][0:64, :],
                        mult)
        # tail: second half's own out projection
        for tt in range(8, 16):
            outproj(tt)


_NC_CACHE = None


def get_nc():
    global _NC_CACHE
    if _NC_CACHE is None:
        _NC_CACHE = build_nc()
    return _NC_CACHE


def make_in_maps(x, attn_mask, Wq, bq, Wk, bk, Wv, bv, Wo, bo):
    import ml_dtypes
    bf = ml_dtypes.bfloat16
    f8 = mybir.dt.np(FP8)
    x = np.asarray(x, np.float32)
    attn_mask = np.asarray(attn_mask)
    in_maps = []
    for core in range(NCORES):
        b, pg = divmod(core, 2)
        cs = slice(pg * CPC, (pg + 1) * CPC)
        xT = np.ascontiguousarray(x[b].T) * WPRE  # [H, L], x16
        x8h = xT.astype(f8)
        x8l = (xT - x8h.astype(np.float32)).astype(f8)
        wvT = np.ascontiguousarray(
            np.asarray(Wv, np.float32)[cs, :].T) * WPRE
        wv8h = wvT.astype(f8)
        wv8l = (wvT - wv8h.astype(np.float32)).astype(f8)
        wqT = np.ascontiguousarray(
            np.asarray(Wq, np.float32)[cs, :].T) * WPRE
        wq8h = wqT.astype(f8)
        wq8l = (wqT - wq8h.astype(np.float32)).astype(f8)
        wkT = np.ascontiguousarray(
            np.asarray(Wk, np.float32)[cs, :].T) * WPRE
        wk8h = wkT.astype(f8)
        wk8l = (wkT - wk8h.astype(np.float32)).astype(f8)
        m = {
            "x8h": x8h,
            "x8l": x8l,
            "wq8h": wq8h, "wq8l": wq8l,
            "wk8h": wk8h, "wk8l": wk8l,
            "wv8h": wv8h,
            "wv8l": wv8l,
            "woT": np.ascontiguousarray(
                np.asarray(Wo, np.float32)[:, cs].T).astype(bf),
            "maskT": np.ascontiguousarray(attn_mask[b, 0].T).astype(bf),
        }
        qkb = np.zeros((128, 2 * PAIRS), np.float32)
        for mc in range(PAIRS):
            chs = slice(pg * CPC + mc * 128, pg * CPC + (mc + 1) * 128)
            qkb[:, mc] = np.asarray(bq, np.float32)[chs] * (WPRE * WPRE)
            qkb[:, PAIRS + mc] = (np.asarray(bk, np.float32)[chs]
                                  * (WPRE * WPRE))
        m["qkb"] = qkb
        in_maps.append(m)
    return in_maps


def gather(results, bo_eff):
    out = np.empty((B, L, H), np.float32)
    for b in range(B):
        out[b] = (results[2 * b]["out"].astype(np.float32)
                  + results[2 * b + 1]["out"].astype(np.float32)
                  + bo_eff)
    return out


def kernel(x, attn_mask, Wq, bq, Wk, bk, Wv, bv, Wo, bo):
    nc = get_nc()
    in_maps = make_in_maps(x, attn_mask, Wq, bq, Wk, bk, Wv, bv, Wo, bo)
    res = bass_utils.run_bass_kernel_spmd(nc, in_maps,
                                          core_ids=list(range(NCORES)))
    bo_eff = (np.asarray(bo, np.float32)
              + np.asarray(bv, np.float32) @ np.asarray(Wo, np.float32).T)
    return gather(res.results, bo_eff)
